# revision 31
# baseline (speedup 1.0000x reference)
"""BartAttention (focused-attention variant) Trainium2 Bass kernel, v2.

Problem (hardcoded): B=2, T=2048, D=1024, H=16 heads, hd=64.
  q = (h @ Wq.T + bq) * hd**-0.5 ; k = h @ Wk.T + bk ; v = h @ Wv.T + bv
  scores = q @ k.T per head ; e = f * exp(scores) ; attn = e / rowsum(e)
  out = (attn @ v) @ Wo.T + bo

Sharding over 8 cores: batch (2) x head-group (4 groups of 4 heads).
Each core computes its heads' QKV, attention, and a partial out-projection
(contraction over its 256 d-columns of Wo); host sums the 4 bf16 partials
per batch in f32 and adds bo.

v2 design notes (all per core):
  - Scores run on the PE in fp8e4 DoubleRow mode with BOTH operands split
    into hi+lo fp8 pairs: lhsT = [k_hi;k_lo] on the 128 partitions
    (duplicated across the two DoubleRow k-subtiles), rhs subtile 0 =
    [q_hi;q_hi], subtile 1 = [q_lo;q_lo].  One DR matmul then computes
    (k_hi+k_lo).T @ (q_hi+q_lo) exactly -- bf16-level precision at 0.5
    cycles/output-column (2x bf16).  q is computed at 8x scale (folded into
    Wq/bq on host) so its fp8 range matches k's; the exp activation applies
    scale=1/16 (1/8 for the q-scale, 1/2 for the hi+lo... no: 1/8 only).
  - e = f * exp(scores) stays bf16.  exp on ACT from [128,1024] PSUM tiles
    (ACT is the bottleneck engine at ~1ns/row); the f-multiply runs on DVE
    with a tunable fraction of tiles offloaded to GPSIMD.
  - PV uses the stationary-swap: lhsT = e[s,t-tile] (stationary), rhs =
    v_aug[s, 65] (moving, ones column 64 gives the rowsum), accumulating
    out[t, 65] per t-tile.  Normalization is then a per-partition scalar
    multiply; a PE transpose puts the result back into [d, t] for the
    out-projection.
  - QKV and out-proj matmuls stay bf16.
"""

import numpy as np
import ml_dtypes

import concourse.bass as bass
import concourse.bacc as bacc
import concourse.mybir as mybir
from concourse.tile import TileContext
from concourse.bass_utils import run_bass_kernel_spmd
from concourse.masks import make_identity

BF16 = mybir.dt.bfloat16
F32 = mybir.dt.float32
FP8 = mybir.dt.float8e4
AF = mybir.ActivationFunctionType
ALU = mybir.AluOpType
DR = mybir.MatmulPerfMode.DoubleRow

B, T, D = 2, 2048, 1024
H, HD = 16, 64
HG = 4               # heads per core
R = HG * HD          # 256 d-rows per core
SCALING = HD ** -0.5
QSCALE = 8.0         # q computed at 8x for better fp8 range; exp scale undoes
N_CORES = 8

P = 128
KT = D // P          # 8 k-tiles for QKV contraction
MT = R // P          # 2 m-tiles (head pairs)
ST = T // P          # 16 s-tiles
NCH = T // 512       # 4 QKV chunks of 512
TW = 1024            # attention t-chunk width (exp tile width)
TCH = T // TW        # 2 attention t-chunks

FMUL_GPS_MOD = 4     # every 5th f-mul tile goes to GPSIMD (0 = none)
OSB_GPS = False       # out-proj PSUM->SBUF copies on GPSIMD
PV_LAG = 2           # units of PV deferral (PE pipelining depth)


def build_bass():
    nc = bacc.Bacc()

    hT_d = nc.declare_dram_parameter("hT", [D, T], BF16, isOutput=False)
    fT_d = nc.declare_dram_parameter("fT", [T, T], BF16, isOutput=False)
    wqT_d = nc.declare_dram_parameter("wqT", [D, R], BF16, isOutput=False)
    wkT_d = nc.declare_dram_parameter("wkT", [D, R], BF16, isOutput=False)
    wvT_d = nc.declare_dram_parameter("wvT", [D, R], BF16, isOutput=False)
    woT_d = nc.declare_dram_parameter("woT", [R, D], BF16, isOutput=False)
    bq_d = nc.declare_dram_parameter("bq", [R, 1], F32, isOutput=False)
    bk_d = nc.declare_dram_parameter("bk", [R, 1], F32, isOutput=False)
    bv_d = nc.declare_dram_parameter("bv", [1, R], BF16, isOutput=False)
    out_d = nc.declare_dram_parameter("out_partial", [T, D], BF16, isOutput=True)

    with TileContext(nc) as tc:
        with (
            nc.allow_low_precision(reason="bf16/fp8 pipeline is intentional"),
            tc.tile_pool(name="sb", bufs=1) as sb,
            tc.tile_pool(name="ps", bufs=1, space="PSUM") as ps,
        ):
            # ---- persistent SBUF tensors ----
            fT = sb.tile([P, ST, T], BF16)
            wqT = sb.tile([P, KT, R], BF16)
            wkT = sb.tile([P, KT, R], BF16)
            wvT = sb.tile([P, KT, R], BF16)
            woT = sb.tile([P, MT, D], BF16)
            bq = sb.tile([P, MT], F32)
            bk = sb.tile([P, MT], F32)
            bv = sb.tile([1, R], BF16)
            ones_r = sb.tile([1, P], BF16)       # K=1 lhsT for v-bias matmul
            ident = sb.tile([P, P], BF16)
            # hi/lo fp8 pairs in m-tile (head-pair) layout
            qhi = sb.tile([P, MT, T], FP8)
            qlo = sb.tile([P, MT, T], FP8)
            khi = sb.tile([P, MT, T], FP8)
            klo = sb.tile([P, MT, T], FP8)
            # per-head DoubleRow operand layouts (partition/subtile dups)
            qm = sb.tile([P, HG, 2, T], FP8)     # [q_hi;q_hi] | [q_lo;q_lo]
            km = sb.tile([P, HG, 2, T], FP8)     # [k_hi;k_lo] duplicated
            vsb = sb.tile([P, ST, HG, HD + 1], BF16)
            po = sb.tile([P, MT, T], BF16)       # out-proj lhsT [256 d, T]

            nc.sync.dma_start(wqT[:], wqT_d.rearrange("(k p) r -> p k r", p=P))
            nc.sync.dma_start(wkT[:], wkT_d.rearrange("(k p) r -> p k r", p=P))
            nc.sync.dma_start(bq[:], bq_d.rearrange("(m p) one -> p (m one)", p=P))
            nc.sync.dma_start(bk[:], bk_d.rearrange("(m p) one -> p (m one)", p=P))
            nc.sync.dma_start(bv[:], bv_d[:])
            nc.vector.memset(ones_r[:], 1.0)
            nc.vector.memset(vsb[:, :, :, HD : HD + 1], 1.0)
            make_identity(nc, ident[:])

            # fT loads are emitted piecewise inside the QKV chunks (behind
            # each hT load) so the 23us transfer doesn't gate QKV startup
            fT_r = fT_d.rearrange("(s p) t -> p s t", p=P)
            hT_r = hT_d.rearrange("(k p) t -> p k t", p=P)
            fmul_i = [0]

            # ---------------- QKV ----------------
            def qkv_thunks(n):
                """Work items for t/s columns [n*512, (n+1)*512), emittable
                interleaved with attention units."""
                nsl = slice(n * 512, (n + 1) * 512)
                thunks = []
                hT = sb.tile([P, KT, 512], BF16, tag="ht", bufs=2,
                             name=f"ht_{n}")

                def hdma():
                    for kk in range(0, KT, 4):
                        nc.sync.dma_start(
                            hT[:, kk : kk + 4, :], hT_r[:, kk : kk + 4, nsl]
                        )
                    if n == 0:
                        nc.sync.dma_start(
                            wvT[:], wvT_d.rearrange("(k p) r -> p k r", p=P)
                        )
                    st4 = 4 * n
                    nc.sync.dma_start(
                        fT[:, st4 : st4 + 4, :], fT_r[:, st4 : st4 + 4, :]
                    )
                thunks.append(hdma)

                def qk_half(j, w_sb, b_sb, hi, lo, half, accbox):
                    if half == 0:
                        accbox[0] = ps.tile([P, 512], F32, tag="op", bufs=2,
                                            name=f"qkacc_{n}_{j}")
                    acc = accbox[0]
                    for k in range(4 * half, 4 * half + 4):
                        nc.tensor.matmul(
                            acc[:],
                            w_sb[:, k, j * P : (j + 1) * P],
                            hT[:, k, :],
                            start=(k == 0),
                            stop=(k == KT - 1),
                        )
                    if half == 1:
                        nc.vector.tensor_scalar_add(
                            hi[:, j, nsl], acc[:], b_sb[:, j : j + 1]
                        )
                        nc.vector.scalar_tensor_tensor(
                            lo[:, j, nsl], acc[:], b_sb[:, j : j + 1],
                            hi[:, j, nsl], op0=ALU.add, op1=ALU.subtract,
                        )

                for j in range(MT):
                    for args in ((wqT, bq, qhi, qlo), (wkT, bk, khi, klo)):
                        box = [None]
                        for half in range(2):
                            thunks.append(
                                lambda j=j, a=args, hf=half, b=box: qk_half(
                                    j, *a, hf, b
                                )
                            )

                def v_group(s):
                    # padded to [P, 512] so the tag size matches the other
                    # "op" tiles; only [:, 0:R] is used
                    acc = ps.tile([P, 512], F32, tag="op", bufs=2, name=f"vacc_{s}")
                    sl = slice((s % 4) * P, (s % 4 + 1) * P)
                    for k in range(KT):
                        nc.tensor.matmul(
                            acc[:, 0:R],
                            hT[:, k, sl],
                            wvT[:, k, :],
                            start=(k == 0),
                            stop=False,
                        )
                    nc.tensor.matmul(
                        acc[:, 0:R], ones_r[:], bv[:], start=False, stop=True
                    )
                    nc.vector.tensor_copy(
                        vsb[:, s, :, 0:HD],
                        acc[:, 0:R].rearrange("p (h d) -> p h d", h=HG),
                    )

                for s in range(4 * n, 4 * n + 4):
                    thunks.append(lambda s=s: v_group(s))

                def kdup():
                    # k DoubleRow layout for this chunk's s-range: partitions
                    # 0:64 = k_hi, 64:128 = k_lo, subtile dup via stride-0
                    for h in range(HG):
                        j, half = h // 2, (h % 2) * HD
                        hr = slice(half, half + HD)
                        nc.sync.dma_start(
                            km[0:HD, h, :, nsl],
                            khi[hr, j, nsl].unsqueeze(1).broadcast_to([HD, 2, 512]),
                        )
                        nc.sync.dma_start(
                            km[HD:P, h, :, nsl],
                            klo[hr, j, nsl].unsqueeze(1).broadcast_to([HD, 2, 512]),
                        )
                thunks.append(kdup)
                return thunks

            def qdup_half(hf):
                """q DoubleRow layout for t columns [hf*1024, (hf+1)*1024):
                subtile 0 = [q_hi; q_hi], subtile 1 = [q_lo; q_lo]."""
                sl = slice(hf * 1024, (hf + 1) * 1024)
                for h in range(HG):
                    j, half = h // 2, (h % 2) * HD
                    hr = slice(half, half + HD)
                    for dst_half in range(2):
                        dsl = slice(dst_half * HD, dst_half * HD + HD)
                        nc.sync.dma_start(qm[dsl, h, 0, sl], qhi[hr, j, sl])
                        nc.sync.dma_start(qm[dsl, h, 1, sl], qlo[hr, j, sl])

            # ---------------- attention ----------------
            pending_pv = []
            deferred_pv = {}

            def attn_unit(h, tch, st, pv_pair, suppress_pv=False):
                """Scores (fp8 DR) -> exp -> f-mul for (head, t-chunk, s-tile);
                the PV matmuls are deferred PV_LAG units for PE pipelining.
                suppress_pv defers them entirely (early-start units whose pv
                slots are still held by an unfinished predecessor head)."""
                ssl = slice(st * P, (st + 1) * P)
                sc = ps.tile([P, TW], F32, tag="sc", bufs=2,
                             name=f"sc_{h}_{tch}_{st}")
                e = sb.tile([P, TW], BF16, tag="e", bufs=16,
                            name=f"e_{h}_{tch}_{st}")
                for a in range(TW // 512):
                    tsl = slice(tch * TW + a * 512, tch * TW + (a + 1) * 512)
                    nc.tensor.matmul(
                        sc[:, a * 512 : (a + 1) * 512],
                        km[:, h, :, ssl],
                        qm[:, h, :, tsl],
                        start=True,
                        stop=True,
                        perf_mode=DR,
                    )
                nc.scalar.activation(e[:], sc[:], AF.Exp, scale=1.0 / QSCALE)
                fmul_i[0] += 1
                eng = (
                    nc.gpsimd
                    if FMUL_GPS_MOD and fmul_i[0] % FMUL_GPS_MOD == 0
                    else nc.vector
                )
                eng.tensor_mul(
                    e[:], e[:], fT[:, st, tch * TW : (tch + 1) * TW]
                )
                if suppress_pv:
                    deferred_pv.setdefault((h, tch), []).append(
                        (h, st, e, pv_pair)
                    )
                else:
                    pending_pv.append((h, st, e, pv_pair))
                    while len(pending_pv) > PV_LAG:
                        flush_one_pv()

            def flush_one_pv():
                h, st, e, pv_pair = pending_pv.pop(0)
                for j4 in range(TW // P):
                    # start=True pending-zeroes the whole 2KB PSUM bank, so
                    # only the first slice of each pv bank may issue it; the
                    # other slices' first writes land on pending-zero bytes
                    # and are replaced (not accumulated) anyway.
                    nc.tensor.matmul(
                        pv_pair[j4 // 4][:, j4 % 4, :],
                        e[:, j4 * P : (j4 + 1) * P],
                        vsb[:, st, h, :],
                        start=(st == 0 and j4 % 4 == 0),
                        stop=(st == ST - 1),
                        skip_group_check=True,
                    )

            def flush_pv():
                while pending_pv:
                    flush_one_pv()

            def norm_head(h, tch, pvt):
                """recip of rowsum col; scalar-mul; PE transpose back to [d,t];
                copy into po."""
                pv_pair, tp = pvt[:2], pvt[2]
                rc = sb.tile([P, 8], F32, tag="rc", bufs=2, name=f"rc_{h}_{tch}")
                for half in range(2):
                    nc.vector.reciprocal(
                        rc[:, half * 4 : half * 4 + 4],
                        pv_pair[half][:, :, HD : HD + 1].rearrange("p f one -> p (f one)"),
                    )
                # odd heads land on partitions 64:128 so the PSUM->SBUF copy
                # stays lane-aligned with po's row range
                pb = (h % 2) * HD
                psl = slice(pb, pb + HD)
                for j4 in range(TW // P):
                    pn = sb.tile([P, HD], BF16, tag="pn", bufs=4,
                                 name=f"pn_{h}_{tch}_{j4}")
                    nc.vector.tensor_scalar_mul(
                        pn[:], pv_pair[j4 // 4][:, j4 % 4, 0:HD],
                        rc[:, j4 : j4 + 1],
                    )
                    nc.tensor.transpose(tp[psl, j4, :], pn[:], ident[:])
                nc.vector.tensor_copy(
                    po[psl, h // 2, tch * TW : (tch + 1) * TW],
                    tp[psl, :, :].rearrange("p a b -> p (a b)"),
                )

            def new_pv_pair(h, tch):
                """PV accumulators + the transpose-staging tile, allocated
                together so the pv-tag slot rotation stays in lockstep."""
                pvt = [
                    ps.tile([P, 4, HD + 1], F32, tag="pv", bufs=2,
                            name=f"pv_{h}_{tch}_{i}")
                    for i in range(2)
                ]
                tp = ps.tile([P, 8, P], BF16, tag="pv", bufs=2,
                             name=f"tp_{h}_{tch}")
                pvt.append(tp)
                return pvt

            def outproj_unit(tt):
                osb = sb.tile([P, D], BF16, tag="osb", bufs=3, name=f"osb_{tt}")
                for nh in range(2):
                    fin = ps.tile([P, 512], F32, tag="op", bufs=2,
                                  name=f"fin_{tt}_{nh}")
                    for j in range(MT):
                        nc.tensor.matmul(
                            fin[:],
                            po[:, j, tt * P : (tt + 1) * P],
                            woT[:, j, nh * 512 : (nh + 1) * 512],
                            start=(j == 0),
                            stop=(j == MT - 1),
                        )
                    # GPSIMD cannot read PSUM: copies stay on DVE
                    nc.vector.tensor_copy(osb[:, nh * 512 : (nh + 1) * 512], fin[:])
                nc.sync.dma_start(out_d[tt * P : (tt + 1) * P, :], osb[:])

            # ---------------- emission ----------------
            # QKV chunks 0,1 first (h0's tch0 scores need q over t 0:1024 and
            # k/v/km s-coverage that grows with chunks); chunks 2,3 are then
            # interleaved ~1:1 with h0's attention units so ACT starts early.
            for th in qkv_thunks(0):
                th()
            for th in qkv_thunks(1):
                th()
            qdup_half(0)

            pv00 = new_pv_pair(0, 0)
            pv10 = new_pv_pair(1, 0)
            pv20 = new_pv_pair(2, 0)
            rest = qkv_thunks(2) + qkv_thunks(3)
            ui0, ui1, ui2 = 0, 0, 0
            H1_EARLY, H2_EARLY = 8, 6
            kd2 = len(rest) // 2 - 1      # chunk 2's kdup index
            for i, th in enumerate(rest):
                th()
                # h0 st8..11 need chunk 2's kdup; st12..15 chunk 3's
                max_ui = 8 if i < kd2 else (12 if i < len(rest) - 1 else ST)
                if ui0 < max_ui:
                    attn_unit(0, 0, ui0, pv00)
                    ui0 += 1
                elif ui1 < H1_EARLY:
                    attn_unit(1, 0, ui1, pv10, suppress_pv=True)
                    ui1 += 1
                elif ui2 < H2_EARLY:
                    attn_unit(2, 0, ui2, pv20, suppress_pv=True)
                    ui2 += 1
            nc.sync.dma_start(woT[:], woT_d.rearrange("(m p) d -> p m d", p=P))
            qdup_half(1)
            while ui0 < ST:
                attn_unit(0, 0, ui0, pv00)
                ui0 += 1

            op_next = [0]

            def emit_outproj(kmax):
                while op_next[0] < kmax:
                    outproj_unit(op_next[0])
                    op_next[0] += 1

            def start_head(h, tch, pvp, first_st, prev3):
                """Emit this head's first unit (pv deferred), drain and norm
                the previous head, then adopt the deferred backlog so it
                flushes behind the norm."""
                attn_unit(h, tch, first_st, pvp, suppress_pv=True)
                while pending_pv:
                    flush_one_pv()
                norm_head(*prev3)
                pending_pv.extend(deferred_pv.pop((h, tch), []))

            prev3 = (0, 0, pv00)
            starts = {(0, 1): ui1, (0, 2): ui2}
            pairs = {(0, 1): pv10, (0, 2): pv20}
            for tch in range(TCH):
                for h in range(HG):
                    if tch == 0 and h == 0:
                        continue
                    pvp = pairs.get((tch, h)) or new_pv_pair(h, tch)
                    first = starts.get((tch, h), 0)
                    start_head(h, tch, pvp, first, prev3)
                    for st in range(first + 1, ST):
                        attn_unit(h, tch, st, pvp)
                        if tch == 1 and h >= 1 and st % 4 == 3:
                            # tt 0..7 (tch0) spread over h1..h3 of tch1
                            emit_outproj(min(8, (h - 1) * 4 + st // 4 + 1))
                    prev3 = (h, tch, pvp)
            flush_pv()
            norm_head(3, 1, prev3[2])
            emit_outproj(16)

    return nc


_NC = None
_LAST_RESULT = None


def _get_nc():
    global _NC
    if _NC is None:
        _NC = build_bass()
        if not _NC.is_finalized():
            _NC.finalize()
    return _NC


def kernel(hidden_states, focused_attention, Wq, bq, Wk, bk, Wv, bv, Wo, bo):
    bf = ml_dtypes.bfloat16
    hT = [np.ascontiguousarray(hidden_states[b].T).astype(bf) for b in range(B)]
    fT = [np.ascontiguousarray(focused_attention[b].T).astype(bf) for b in range(B)]

    in_maps = []
    for c in range(N_CORES):
        b, g = divmod(c, 4)
        rows = slice(g * R, (g + 1) * R)
        in_maps.append({
            "hT": hT[b],
            "fT": fT[b],
            "wqT": np.ascontiguousarray((Wq[rows] * (SCALING * QSCALE)).T).astype(bf),
            "wkT": np.ascontiguousarray(Wk[rows].T).astype(bf),
            "wvT": np.ascontiguousarray(Wv[rows].T).astype(bf),
            "woT": np.ascontiguousarray(Wo[:, rows].T).astype(bf),
            "bq": np.ascontiguousarray(
                (bq[rows] * (SCALING * QSCALE))[:, None]
            ).astype(np.float32),
            "bk": np.ascontiguousarray(bk[rows][:, None]).astype(np.float32),
            "bv": np.ascontiguousarray(bv[rows][None, :]).astype(bf),
        })

    res = run_bass_kernel_spmd(_get_nc(), in_maps, list(range(N_CORES)))
    global _LAST_RESULT
    _LAST_RESULT = res
    out = np.zeros((B, T, D), dtype=np.float32)
    for c in range(N_CORES):
        out[c // 4] += np.asarray(res.results[c]["out_partial"], dtype=np.float32)
    out += np.asarray(bo, dtype=np.float32)[None, None, :]
    return out


# revision 71
# speedup vs baseline: 1.2265x; 1.2265x over previous
"""BartAttention (focused-attention variant) Trainium2 Bass kernel, v2.

Problem (hardcoded): B=2, T=2048, D=1024, H=16 heads, hd=64.
  q = (h @ Wq.T + bq) * hd**-0.5 ; k = h @ Wk.T + bk ; v = h @ Wv.T + bv
  scores = q @ k.T per head ; e = f * exp(scores) ; attn = e / rowsum(e)
  out = (attn @ v) @ Wo.T + bo

Sharding over 8 cores: batch (2) x head-group (4 groups of 4 heads).
Each core computes its heads' QKV, attention, and a partial out-projection
(contraction over its 256 d-columns of Wo); host sums the 4 bf16 partials
per batch in f32 and adds bo.

Design notes (all per core):
  - Scores run on the PE in fp8e4 DoubleRow mode with BOTH operands split
    into hi+lo fp8 pairs: lhsT = [k_hi;k_lo] on the 128 partitions (the
    DoubleRow k-subtile dim is a stride-0 broadcast), rhs subtile 0 =
    [q_hi;q_hi], subtile 1 = [q_lo;q_lo].  One DR matmul computes
    (k_hi+k_lo).T @ (q_hi+q_lo) exactly -- bf16-level precision at 0.5
    cycles/output-column (2x bf16, 4x the naive K=64 layout).  q is
    computed at 8x scale (folded into Wq/bq on host) for fp8 range; the
    exp activation's scale=1/8 undoes it.
  - e = f * exp(scores) stays bf16: exp on ACT from [128,1024] PSUM tiles
    (ACT is the global bottleneck at ~1ns/row, ~133us/core), f-multiply
    on DVE in 2x mode.  PSUM start=True pending-zeroes a whole 2KB bank,
    so only the first slice of each pv bank issues it.
  - PV uses the stationary-swap: lhsT = e[s, t-tile] (stationary), rhs =
    v_aug[s, 65] (moving; ones column 64 gives the rowsum), accumulating
    out[t, 65] per t-tile at 65 cols/matmul.  Normalization is a
    broadcast multiply by 1/rowsum per t-partition; a PE transpose puts
    the result back into [d, t] for the out-projection.
  - QKV (m-tile-major so head-pair 0 finishes first and attention starts
    ~17us in) and out-proj stay bf16.  The emission interleaves QKV
    thunks with gated attention units, runs h1 through the stream tail,
    and early-starts h2 with its PV matmuls deferred.
"""

import numpy as np
import ml_dtypes

import concourse.bass as bass
import concourse.bacc as bacc
import concourse.mybir as mybir
from concourse.tile import TileContext
from concourse.bass_utils import run_bass_kernel_spmd
from concourse.masks import make_identity

BF16 = mybir.dt.bfloat16
F32 = mybir.dt.float32
FP8 = mybir.dt.float8e4
AF = mybir.ActivationFunctionType
ALU = mybir.AluOpType
DR = mybir.MatmulPerfMode.DoubleRow

B, T, D = 2, 2048, 1024
H, HD = 16, 64
HG = 4               # heads per core
R = HG * HD          # 256 d-rows per core
SCALING = HD ** -0.5
QSCALE = 8.0         # q computed at 8x for better fp8 range; exp scale undoes
N_CORES = 8

P = 128
KT = D // P          # 8 k-tiles for QKV contraction
MT = R // P          # 2 m-tiles (head pairs)
ST = T // P          # 16 s-tiles
NCH = T // 512       # 4 QKV chunks of 512
TW = 1024            # attention t-chunk width (exp tile width)
TCH = T // TW        # 2 attention t-chunks

FMUL_GPS_MOD = 12     # every 5th f-mul tile goes to GPSIMD (0 = none)
OSB_GPS = False       # out-proj PSUM->SBUF copies on GPSIMD
PV_LAG = 6           # units of PV deferral (PE pipelining depth)


def build_bass():
    nc = bacc.Bacc()

    hT_d = nc.declare_dram_parameter("hT", [D, T], BF16, isOutput=False)
    fT_d = nc.declare_dram_parameter("fT", [T, T], BF16, isOutput=False)
    wqT_d = nc.declare_dram_parameter("wqT", [D, R], BF16, isOutput=False)
    wkT_d = nc.declare_dram_parameter("wkT", [D, R], BF16, isOutput=False)
    wvT_d = nc.declare_dram_parameter("wvT", [D, R], BF16, isOutput=False)
    woT_d = nc.declare_dram_parameter("woT", [R, D], BF16, isOutput=False)
    bq_d = nc.declare_dram_parameter("bq", [R, 1], F32, isOutput=False)
    bk_d = nc.declare_dram_parameter("bk", [R, 1], F32, isOutput=False)
    bv_d = nc.declare_dram_parameter("bv", [1, R], BF16, isOutput=False)
    out_d = nc.declare_dram_parameter("out_partial", [T, D], BF16, isOutput=True)

    with TileContext(nc) as tc:
        with (
            nc.allow_low_precision(reason="bf16/fp8 pipeline is intentional"),
            tc.tile_pool(name="sb", bufs=1) as sb,
            tc.tile_pool(name="ps", bufs=1, space="PSUM") as ps,
        ):
            # ---- persistent SBUF tensors ----
            fT = sb.tile([P, ST, T], BF16)
            wqT = sb.tile([P, KT, R], BF16)
            wkT = sb.tile([P, KT, R], BF16)
            wvT = sb.tile([P, KT, R], BF16)
            woT = sb.tile([P, MT, D], BF16)
            bq = sb.tile([P, MT], F32)
            bk = sb.tile([P, MT], F32)
            bv = sb.tile([1, R], BF16)
            ones_r = sb.tile([1, P], BF16)       # K=1 lhsT for v-bias matmul
            ident = sb.tile([P, P], BF16)
            # hi/lo fp8 pairs in m-tile (head-pair) layout
            qhi = sb.tile([P, MT, T], FP8)
            qlo = sb.tile([P, MT, T], FP8)
            khi = sb.tile([P, MT, T], FP8)
            klo = sb.tile([P, MT, T], FP8)
            # per-head DoubleRow operand layouts (partition/subtile dups)
            qm = sb.tile([P, HG, 2, T], FP8)     # [q_hi;q_hi] | [q_lo;q_lo]
            km = sb.tile([P, HG, T], FP8)        # [k_hi;k_lo] (subtile dim
                                                 # broadcast in the matmul)
            vsb = sb.tile([P, ST, HG, HD + 1], BF16)
            po = sb.tile([P, MT, T], BF16)       # out-proj lhsT [256 d, T]

            nc.sync.dma_start(wqT[:], wqT_d.rearrange("(k p) r -> p k r", p=P))
            nc.sync.dma_start(wkT[:], wkT_d.rearrange("(k p) r -> p k r", p=P))
            nc.sync.dma_start(bq[:], bq_d.rearrange("(m p) one -> p (m one)", p=P))
            nc.sync.dma_start(bk[:], bk_d.rearrange("(m p) one -> p (m one)", p=P))
            nc.sync.dma_start(bv[:], bv_d[:])
            nc.vector.memset(ones_r[:], 1.0)
            nc.vector.memset(vsb[:, :, :, HD : HD + 1], 1.0)
            make_identity(nc, ident[:])

            # fT loads are emitted piecewise inside the QKV chunks (behind
            # each hT load) so the 23us transfer doesn't gate QKV startup
            fT_r = fT_d.rearrange("(s p) t -> p s t", p=P)
            hT_r = hT_d.rearrange("(k p) t -> p k t", p=P)
            fmul_i = [0]

            # ---------------- QKV (m-tile-major) ----------------
            # head-pair 0's full-T q/k complete first so attention starts
            # ~16us in and runs continuously while head-pair 1's QKV streams
            hTsb = sb.tile([P, KT, T], BF16)
            accboxes = {}

            def hdma(n):
                nsl = slice(n * 512, (n + 1) * 512)
                for kk in range(0, KT, 4):
                    nc.sync.dma_start(
                        hTsb[:, kk : kk + 4, nsl], hT_r[:, kk : kk + 4, nsl]
                    )
                if n == 0:
                    nc.sync.dma_start(
                        wvT[:], wvT_d.rearrange("(k p) r -> p k r", p=P)
                    )

            def ft_piece(g):
                nc.sync.dma_start(
                    fT[:, 4 * g : 4 * g + 4, :], fT_r[:, 4 * g : 4 * g + 4, :]
                )

            def qk_half(j, n, qk, half):
                w_sb, b_sb, hi, lo = (
                    (wqT, bq, qhi, qlo) if qk == 0 else (wkT, bk, khi, klo)
                )
                nsl = slice(n * 512, (n + 1) * 512)
                key = (j, n, qk)
                if half == 0:
                    accboxes[key] = ps.tile([P, 512], F32, tag="op", bufs=2,
                                            name=f"qkacc_{j}_{n}_{qk}")
                acc = accboxes[key]
                for k in range(4 * half, 4 * half + 4):
                    nc.tensor.matmul(
                        acc[:],
                        w_sb[:, k, j * P : (j + 1) * P],
                        hTsb[:, k, nsl],
                        start=(k == 0),
                        stop=(k == KT - 1),
                    )
                if half == 1:
                    nc.vector.tensor_scalar_add(
                        hi[:, j, nsl], acc[:], b_sb[:, j : j + 1]
                    )
                    nc.vector.scalar_tensor_tensor(
                        lo[:, j, nsl], acc[:], b_sb[:, j : j + 1],
                        hi[:, j, nsl], op0=ALU.add, op1=ALU.subtract,
                    )

            def v_group(s):
                # padded to [P, 512] so the tag size matches the other
                # "op" tiles; only [:, 0:R] is used
                acc = ps.tile([P, 512], F32, tag="op", bufs=2, name=f"vacc_{s}")
                for k in range(KT):
                    nc.tensor.matmul(
                        acc[:, 0:R],
                        hTsb[:, k, s * P : (s + 1) * P],
                        wvT[:, k, :],
                        start=(k == 0),
                        stop=False,
                    )
                nc.tensor.matmul(
                    acc[:, 0:R], ones_r[:], bv[:], start=False, stop=True
                )
                nc.scalar.copy(
                    vsb[:, s, :, 0:HD],
                    acc[:, 0:R].rearrange("p (h d) -> p h d", h=HG),
                )

            def kdup_pair(j, n):
                # k DoubleRow layout for head-pair j, s-chunk n: partitions
                # 0:64 = k_hi, 64:128 = k_lo, subtile dup via stride-0
                nsl = slice(n * 512, (n + 1) * 512)
                for h in (2 * j, 2 * j + 1):
                    hr = slice((h % 2) * HD, (h % 2) * HD + HD)
                    nc.sync.dma_start(km[0:HD, h, nsl], khi[hr, j, nsl])
                    nc.sync.dma_start(km[HD:P, h, nsl], klo[hr, j, nsl])

            def qdup_pair(j, hf):
                # q DoubleRow layout for head-pair j, t columns
                # [hf*1024, (hf+1)*1024): subtile 0 = [q_hi; q_hi], 1 = lo
                sl = slice(hf * 1024, (hf + 1) * 1024)
                for h in (2 * j, 2 * j + 1):
                    hr = slice((h % 2) * HD, (h % 2) * HD + HD)
                    for dst_half in range(2):
                        dsl = slice(dst_half * HD, dst_half * HD + HD)
                        nc.sync.dma_start(qm[dsl, h, 0, sl], qhi[hr, j, sl])
                        nc.sync.dma_start(qm[dsl, h, 1, sl], qlo[hr, j, sl])

            # ---------------- attention ----------------
            pending_pv = []
            deferred_pv = {}

            def attn_unit(h, tch, st, pv_pair, suppress_pv=False):
                """Scores (fp8 DR) -> exp -> f-mul for (head, t-chunk, s-tile);
                the PV matmuls are deferred PV_LAG units for PE pipelining.
                suppress_pv defers them entirely (early-start units whose pv
                slots are still held by an unfinished predecessor head)."""
                ssl = slice(st * P, (st + 1) * P)
                sc = ps.tile([P, TW], F32, tag="sc", bufs=2,
                             name=f"sc_{h}_{tch}_{st}")
                e = sb.tile([P, TW], BF16, tag="e", bufs=15,
                            name=f"e_{h}_{tch}_{st}")
                for a in range(TW // 512):
                    tsl = slice(tch * TW + a * 512, tch * TW + (a + 1) * 512)
                    nc.tensor.matmul(
                        sc[:, a * 512 : (a + 1) * 512],
                        km[:, h, ssl].unsqueeze(1).broadcast_to([P, 2, P]),
                        qm[:, h, :, tsl],
                        start=True,
                        stop=True,
                        perf_mode=DR,
                    )
                nc.scalar.activation(e[:], sc[:], AF.Exp, scale=1.0 / QSCALE)
                fmul_i[0] += 1
                # Pool takes a share of f-muls only inside the startup window
                # where DVE is the wall (units ~20-60); elsewhere Pool's slow
                # tiles would sit on the pv critical chain
                eng = (
                    nc.gpsimd
                    if (FMUL_GPS_MOD and 20 <= fmul_i[0] <= 60
                        and fmul_i[0] % FMUL_GPS_MOD == 0)
                    else nc.vector
                )
                eng.tensor_mul(
                    e[:], e[:], fT[:, st, tch * TW : (tch + 1) * TW]
                )
                if suppress_pv:
                    deferred_pv.setdefault((h, tch), []).append(
                        (h, st, e, pv_pair)
                    )
                else:
                    pending_pv.append((h, st, e, pv_pair))
                    while len(pending_pv) > PV_LAG:
                        flush_one_pv()

            def flush_one_pv():
                h, st, e, pv_pair = pending_pv.pop(0)
                for j4 in range(TW // P):
                    # start=True pending-zeroes the whole 2KB PSUM bank, so
                    # only the first slice of each pv bank may issue it; the
                    # other slices' first writes land on pending-zero bytes
                    # and are replaced (not accumulated) anyway.
                    nc.tensor.matmul(
                        pv_pair[j4 // 4][:, j4 % 4, :],
                        e[:, j4 * P : (j4 + 1) * P],
                        vsb[:, st, h, :],
                        start=(st == 0 and j4 % 4 == 0),
                        stop=(st == ST - 1),
                        skip_group_check=True,
                    )

            def flush_pv():
                while pending_pv:
                    flush_one_pv()

            def norm_head(h, tch, pvt, last=False):
                """recip of rowsum col; scalar-mul; PE transpose back to [d,t];
                copy into po.  For the final head (ACT idle) the muls split
                across ACT and DVE."""
                pv_pair, tp = pvt[:2], pvt[2]
                rc = sb.tile([P, 8], F32, tag="rc", bufs=2, name=f"rc_{h}_{tch}")
                for half in range(2):
                    nc.vector.reciprocal(
                        rc[:, half * 4 : half * 4 + 4],
                        pv_pair[half][:, :, HD : HD + 1].rearrange("p f one -> p (f one)"),
                    )
                # odd heads land on partitions 64:128 so the PSUM->SBUF copy
                # stays lane-aligned with po's row range
                pb = (h % 2) * HD
                psl = slice(pb, pb + HD)
                for half in range(2):
                    pn = sb.tile([P, 4, HD], BF16, tag="pn", bufs=3,
                                 name=f"pn_{h}_{tch}_{half}")
                    nc.vector.tensor_mul(
                        pn[:], pv_pair[half][:, :, 0:HD],
                        rc[:, half * 4 : half * 4 + 4]
                        .unsqueeze(-1).broadcast_to([P, 4, HD]),
                    )
                    for i in range(4):
                        nc.tensor.transpose(
                            tp[psl, half * 4 + i, :], pn[:, i, :], ident[:]
                        )
                if last:
                    for hh in range(2):
                        nc.vector.tensor_copy(
                            po[psl, h // 2,
                               tch * TW + hh * 512 : tch * TW + (hh + 1) * 512],
                            tp[psl, hh * 4 : hh * 4 + 4, :]
                            .rearrange("p a b -> p (a b)"),
                        )
                else:
                    nc.vector.tensor_copy(
                        po[psl, h // 2, tch * TW : (tch + 1) * TW],
                        tp[psl, :, :].rearrange("p a b -> p (a b)"),
                    )

            def new_pv_pair(h, tch):
                """PV accumulators + the transpose-staging tile, allocated
                together so the pv-tag slot rotation stays in lockstep."""
                pvt = [
                    ps.tile([P, 4, HD + 1], F32, tag="pv", bufs=2,
                            name=f"pv_{h}_{tch}_{i}")
                    for i in range(2)
                ]
                tp = ps.tile([P, 8, P], BF16, tag="pv", bufs=2,
                             name=f"tp_{h}_{tch}")
                pvt.append(tp)
                return pvt

            def outproj_unit(tt):
                osb = sb.tile([P, D], BF16, tag="osb", bufs=3, name=f"osb_{tt}")
                tail = tt >= 8
                for nh in range(2):
                    # tail fins alternate between the op tag and the freed
                    # pv banks so four accumulators are in flight
                    ftag = "pv" if (tail and tt % 2 == 1) else "op"
                    finn = ps.tile([P, 512], F32, tag=ftag, bufs=2,
                                   name=f"fin_{tt}_{nh}")
                    for j in range(MT):
                        nc.tensor.matmul(
                            finn[:],
                            po[:, j, tt * P : (tt + 1) * P],
                            woT[:, j, nh * 512 : (nh + 1) * 512],
                            start=(j == 0),
                            stop=(j == MT - 1),
                        )
                    # GPSIMD cannot read PSUM
                    if tail and nh == 0:
                        nc.scalar.copy(osb[:, 0:512], finn[:])
                    else:
                        nc.vector.tensor_copy(
                            osb[:, nh * 512 : (nh + 1) * 512], finn[:]
                        )
                nc.sync.dma_start(out_d[tt * P : (tt + 1) * P, :], osb[:])

            # ---------------- emission ----------------
            pv00 = new_pv_pair(0, 0)
            pv10 = new_pv_pair(1, 0)
            pv20 = new_pv_pair(2, 0)
            H1_EARLY, H2_EARLY = 6, 3

            # phase 0: everything h0's first attention units need (the pv
            # matmuls lag 2 units, so v streams just behind)
            hdma(0)
            hdma(1)
            ft_piece(0)
            for n in (0, 1):
                for qk in (0, 1):
                    for half in (0, 1):
                        qk_half(0, n, qk, half)
            kdup_pair(0, 0)
            kdup_pair(0, 1)
            qdup_pair(0, 0)
            v_group(0)
            v_group(1)

            # stream: remaining QKV work in h0-unlock order, one attention
            # unit after each thunk when its gates have passed
            done = set()
            stream = []

            def ev(th, *events):
                stream.append((th, events))

            ev(lambda: v_group(2), "v2")
            ev(lambda: v_group(3), "v3")
            ev(lambda: v_group(4), "v4")
            ev(lambda: v_group(5), "v5")
            ev(lambda: ft_piece(1), "f1")
            ev(lambda: v_group(6), "v6")
            ev(lambda: v_group(7), "v7")
            ev(lambda: hdma(2))
            ev(lambda: qk_half(0, 2, 1, 0))
            ev(lambda: qk_half(0, 2, 1, 1))
            ev(lambda: kdup_pair(0, 2), "kd2")
            ev(lambda: ft_piece(2), "f2")
            ev(lambda: v_group(8), "v8")
            ev(lambda: v_group(9), "v9")
            ev(lambda: v_group(10), "v10")
            ev(lambda: v_group(11), "v11")
            ev(lambda: hdma(3))
            ev(lambda: qk_half(0, 3, 1, 0))
            ev(lambda: qk_half(0, 3, 1, 1))
            ev(lambda: kdup_pair(0, 3), "kd3")
            ev(lambda: ft_piece(3), "f3")
            ev(lambda: v_group(12), "v12")
            ev(lambda: v_group(13), "v13")
            ev(lambda: v_group(14), "v14")
            ev(lambda: v_group(15), "v15")
            for n in (2, 3):
                for hf in (0, 1):
                    ev(lambda n=n, hf=hf: qk_half(0, n, 0, hf))
            ev(lambda: qdup_pair(0, 1))
            for n in range(NCH):
                for qk in (0, 1):
                    for hf in (0, 1):
                        ev(lambda n=n, qk=qk, hf=hf: qk_half(1, n, qk, hf))
                ev(lambda n=n: kdup_pair(1, n), f"kd1_{n}")
                if n == 1:
                    ev(lambda: qdup_pair(1, 0), "qd1")
            ev(lambda: qdup_pair(1, 1))
            ev(lambda: nc.sync.dma_start(
                woT[:], woT_d.rearrange("(m p) d -> p m d", p=P)))

            def h0_ready(st):
                # the pv matmuls for unit st are emitted PV_LAG units later,
                # so v only needs to be a few steps ahead of the flush
                need = []
                vst = st - (PV_LAG - 1)
                if vst >= 2:
                    need.append(f"v{vst}")
                if st >= 8:
                    need.append("kd2" if st < 12 else "kd3")
                if st >= 4:
                    need.append(f"f{st // 4}")
                return all(x in done for x in need)

            def h2_ready(st):
                return all(x in done for x in ("qd1", f"kd1_{st // 4}"))

            ui0, ui1, ui2 = 0, 0, 0
            for th, events in stream:
                # emit the eligible unit BEFORE the thunk so its scores sit
                # ahead of the thunk's matmuls in the PE queue
                if ui0 < ST and h0_ready(ui0):
                    attn_unit(0, 0, ui0, pv00)
                    ui0 += 1
                elif ui0 >= ST and ui1 < H1_EARLY:
                    attn_unit(1, 0, ui1, pv10, suppress_pv=True)
                    ui1 += 1
                elif (ui0 >= ST and ui1 >= H1_EARLY and ui2 < H2_EARLY
                        and h2_ready(ui2)):
                    attn_unit(2, 0, ui2, pv20, suppress_pv=True)
                    ui2 += 1
            while ui0 < ST:
                attn_unit(0, 0, ui0, pv00)
                ui0 += 1

            op_next = [0]

            def emit_outproj(kmax):
                while op_next[0] < kmax:
                    outproj_unit(op_next[0])
                    op_next[0] += 1

            def start_head(h, tch, pvp, first_st, prev3):
                """Emit this head's first unit (pv deferred), drain and norm
                the previous head, then adopt the deferred backlog so it
                flushes behind the norm."""
                attn_unit(h, tch, first_st, pvp, suppress_pv=True)
                while pending_pv:
                    flush_one_pv()
                norm_head(*prev3)
                pending_pv.extend(deferred_pv.pop((h, tch), []))

            while ui1 < ST:
                attn_unit(1, 0, ui1, pv10)
                ui1 += 1
            prev3 = (1, 0, pv10)
            starts = {(0, 2): ui2}
            pairs = {(0, 2): pv20}
            for tch in range(TCH):
                for h in range(HG):
                    if tch == 0 and h <= 1:
                        continue
                    pvp = pairs.get((tch, h)) or new_pv_pair(h, tch)
                    first = starts.get((tch, h), 0)
                    start_head(h, tch, pvp, first, prev3)
                    for st in range(first + 1, ST):
                        attn_unit(h, tch, st, pvp)
                        if tch == 1 and h == 3:
                            # drain eagerly so the post-exp tail is short
                            while len(pending_pv) > 2:
                                flush_one_pv()
                        if tch == 1 and h < 3 and (h * ST + st) % 5 == 4:
                            # tt 0..7 (tch0) spread over h0..h2 of tch1 --
                            # h3's window must stay clean so the last exps
                            # aren't interleaved with out-proj traffic
                            emit_outproj(min(8, (h * ST + st) // 5 + 1))
                    prev3 = (h, tch, pvp)
            flush_pv()
            norm_head(3, 1, prev3[2], last=True)
            emit_outproj(16)

    return nc


_NC = None
_LAST_RESULT = None


def _get_nc():
    global _NC
    if _NC is None:
        _NC = build_bass()
        if not _NC.is_finalized():
            _NC.finalize()
    return _NC


def kernel(hidden_states, focused_attention, Wq, bq, Wk, bk, Wv, bv, Wo, bo):
    bf = ml_dtypes.bfloat16
    hT = [np.ascontiguousarray(hidden_states[b].T).astype(bf) for b in range(B)]
    fT = [np.ascontiguousarray(focused_attention[b].T).astype(bf) for b in range(B)]

    in_maps = []
    for c in range(N_CORES):
        b, g = divmod(c, 4)
        rows = slice(g * R, (g + 1) * R)
        in_maps.append({
            "hT": hT[b],
            "fT": fT[b],
            "wqT": np.ascontiguousarray((Wq[rows] * (SCALING * QSCALE)).T).astype(bf),
            "wkT": np.ascontiguousarray(Wk[rows].T).astype(bf),
            "wvT": np.ascontiguousarray(Wv[rows].T).astype(bf),
            "woT": np.ascontiguousarray(Wo[:, rows].T).astype(bf),
            "bq": np.ascontiguousarray(
                (bq[rows] * (SCALING * QSCALE))[:, None]
            ).astype(np.float32),
            "bk": np.ascontiguousarray(bk[rows][:, None]).astype(np.float32),
            "bv": np.ascontiguousarray(bv[rows][None, :]).astype(bf),
        })

    res = run_bass_kernel_spmd(_get_nc(), in_maps, list(range(N_CORES)))
    global _LAST_RESULT
    _LAST_RESULT = res
    out = np.zeros((B, T, D), dtype=np.float32)
    for c in range(N_CORES):
        out[c // 4] += np.asarray(res.results[c]["out_partial"], dtype=np.float32)
    out += np.asarray(bo, dtype=np.float32)[None, None, :]
    return out


# revision 77
# speedup vs baseline: 1.2266x; 1.0001x over previous
"""BartAttention (focused-attention variant) Trainium2 Bass kernel, v2.

Problem (hardcoded): B=2, T=2048, D=1024, H=16 heads, hd=64.
  q = (h @ Wq.T + bq) * hd**-0.5 ; k = h @ Wk.T + bk ; v = h @ Wv.T + bv
  scores = q @ k.T per head ; e = f * exp(scores) ; attn = e / rowsum(e)
  out = (attn @ v) @ Wo.T + bo

Sharding over 8 cores: batch (2) x head-group (4 groups of 4 heads).
Each core computes its heads' QKV, attention, and a partial out-projection
(contraction over its 256 d-columns of Wo); host sums the 4 bf16 partials
per batch in f32 and adds bo.

Design notes (all per core):
  - Scores run on the PE in fp8e4 DoubleRow mode with BOTH operands split
    into hi+lo fp8 pairs: lhsT = [k_hi;k_lo] on the 128 partitions (the
    DoubleRow k-subtile dim is a stride-0 broadcast), rhs subtile 0 =
    [q_hi;q_hi], subtile 1 = [q_lo;q_lo].  One DR matmul computes
    (k_hi+k_lo).T @ (q_hi+q_lo) exactly -- bf16-level precision at 0.5
    cycles/output-column (2x bf16, 4x the naive K=64 layout).  q is
    computed at 8x scale (folded into Wq/bq on host) for fp8 range; the
    exp activation's scale=1/8 undoes it.
  - e = f * exp(scores) stays bf16: exp on ACT from [128,1024] PSUM tiles
    (ACT is the global bottleneck at ~1ns/row, ~133us/core), f-multiply
    on DVE in 2x mode.  PSUM start=True pending-zeroes a whole 2KB bank,
    so only the first slice of each pv bank issues it.
  - PV uses the stationary-swap: lhsT = e[s, t-tile] (stationary), rhs =
    v_aug[s, 65] (moving; ones column 64 gives the rowsum), accumulating
    out[t, 65] per t-tile at 65 cols/matmul.  Normalization is a
    broadcast multiply by 1/rowsum per t-partition; a PE transpose puts
    the result back into [d, t] for the out-projection.
  - QKV (m-tile-major so head-pair 0 finishes first and attention starts
    ~17us in) and out-proj stay bf16.  The emission interleaves QKV
    thunks with gated attention units, runs h1 through the stream tail,
    and early-starts h2 with its PV matmuls deferred.
"""

import numpy as np
import ml_dtypes

import concourse.bass as bass
import concourse.bacc as bacc
import concourse.mybir as mybir
from concourse.tile import TileContext
from concourse.bass_utils import run_bass_kernel_spmd
from concourse.masks import make_identity

BF16 = mybir.dt.bfloat16
F32 = mybir.dt.float32
FP8 = mybir.dt.float8e4
AF = mybir.ActivationFunctionType
ALU = mybir.AluOpType
DR = mybir.MatmulPerfMode.DoubleRow

B, T, D = 2, 2048, 1024
H, HD = 16, 64
HG = 4               # heads per core
R = HG * HD          # 256 d-rows per core
SCALING = HD ** -0.5
QSCALE = 8.0         # q computed at 8x for better fp8 range; exp scale undoes
N_CORES = 8

P = 128
KT = D // P          # 8 k-tiles for QKV contraction
MT = R // P          # 2 m-tiles (head pairs)
ST = T // P          # 16 s-tiles
NCH = T // 512       # 4 QKV chunks of 512
TW = 1024            # attention t-chunk width (exp tile width)
TCH = T // TW        # 2 attention t-chunks

FMUL_GPS_MOD = 12     # every 5th f-mul tile goes to GPSIMD (0 = none)
OSB_GPS = False       # out-proj PSUM->SBUF copies on GPSIMD
PV_LAG = 6           # units of PV deferral (PE pipelining depth)


def build_bass():
    nc = bacc.Bacc()

    hT_d = nc.declare_dram_parameter("hT", [D, T], BF16, isOutput=False)
    fT_d = nc.declare_dram_parameter("fT", [T, T], BF16, isOutput=False)
    wqT_d = nc.declare_dram_parameter("wqT", [D, R], BF16, isOutput=False)
    wkT_d = nc.declare_dram_parameter("wkT", [D, R], BF16, isOutput=False)
    wvT_d = nc.declare_dram_parameter("wvT", [D, R], BF16, isOutput=False)
    woT_d = nc.declare_dram_parameter("woT", [R, D], BF16, isOutput=False)
    bq_d = nc.declare_dram_parameter("bq", [R, 1], F32, isOutput=False)
    bk_d = nc.declare_dram_parameter("bk", [R, 1], F32, isOutput=False)
    bv_d = nc.declare_dram_parameter("bv", [1, R], BF16, isOutput=False)
    out_d = nc.declare_dram_parameter("out_partial", [T, D], BF16, isOutput=True)

    with TileContext(nc) as tc:
        with (
            nc.allow_low_precision(reason="bf16/fp8 pipeline is intentional"),
            tc.tile_pool(name="sb", bufs=1) as sb,
            tc.tile_pool(name="ps", bufs=1, space="PSUM") as ps,
        ):
            # ---- persistent SBUF tensors ----
            fT = sb.tile([P, ST, T], BF16)
            wqT = sb.tile([P, KT, R], BF16)
            wkT = sb.tile([P, KT, R], BF16)
            wvT = sb.tile([P, KT, R], BF16)
            woT = sb.tile([P, MT, D], BF16)
            bq = sb.tile([P, MT], F32)
            bk = sb.tile([P, MT], F32)
            bv = sb.tile([1, R], BF16)
            ones_r = sb.tile([1, P], BF16)       # K=1 lhsT for v-bias matmul
            ident = sb.tile([P, P], BF16)
            # hi/lo fp8 pairs in m-tile (head-pair) layout
            qhi = sb.tile([P, MT, T], FP8)
            qlo = sb.tile([P, MT, T], FP8)
            khi = sb.tile([P, MT, T], FP8)
            klo = sb.tile([P, MT, T], FP8)
            # per-head DoubleRow operand layouts (partition/subtile dups)
            qm = sb.tile([P, HG, 2, T], FP8)     # [q_hi;q_hi] | [q_lo;q_lo]
            km = sb.tile([P, HG, T], FP8)        # [k_hi;k_lo] (subtile dim
                                                 # broadcast in the matmul)
            vsb = sb.tile([P, ST, HG, HD + 1], BF16)
            po = sb.tile([P, MT, T], BF16)       # out-proj lhsT [256 d, T]

            nc.sync.dma_start(wqT[:], wqT_d.rearrange("(k p) r -> p k r", p=P))
            nc.sync.dma_start(wkT[:], wkT_d.rearrange("(k p) r -> p k r", p=P))
            nc.sync.dma_start(bq[:], bq_d.rearrange("(m p) one -> p (m one)", p=P))
            nc.sync.dma_start(bk[:], bk_d.rearrange("(m p) one -> p (m one)", p=P))
            nc.sync.dma_start(bv[:], bv_d[:])
            nc.vector.memset(ones_r[:], 1.0)
            nc.vector.memset(vsb[:, :, :, HD : HD + 1], 1.0)
            make_identity(nc, ident[:])

            # fT loads are emitted piecewise inside the QKV chunks (behind
            # each hT load) so the 23us transfer doesn't gate QKV startup
            fT_r = fT_d.rearrange("(s p) t -> p s t", p=P)
            hT_r = hT_d.rearrange("(k p) t -> p k t", p=P)
            fmul_i = [0]

            # ---------------- QKV (m-tile-major) ----------------
            # head-pair 0's full-T q/k complete first so attention starts
            # ~16us in and runs continuously while head-pair 1's QKV streams
            hTsb = sb.tile([P, KT, T], BF16)
            accboxes = {}

            def hdma(n):
                nsl = slice(n * 512, (n + 1) * 512)
                for kk in range(0, KT, 4):
                    nc.sync.dma_start(
                        hTsb[:, kk : kk + 4, nsl], hT_r[:, kk : kk + 4, nsl]
                    )
                if n == 0:
                    nc.sync.dma_start(
                        wvT[:], wvT_d.rearrange("(k p) r -> p k r", p=P)
                    )

            def ft_piece(g):
                nc.sync.dma_start(
                    fT[:, 4 * g : 4 * g + 4, :], fT_r[:, 4 * g : 4 * g + 4, :]
                )

            def qk_half(j, n, qk, half):
                w_sb, b_sb, hi, lo = (
                    (wqT, bq, qhi, qlo) if qk == 0 else (wkT, bk, khi, klo)
                )
                nsl = slice(n * 512, (n + 1) * 512)
                key = (j, n, qk)
                if half == 0:
                    accboxes[key] = ps.tile([P, 512], F32, tag="op", bufs=2,
                                            name=f"qkacc_{j}_{n}_{qk}")
                acc = accboxes[key]
                for k in range(4 * half, 4 * half + 4):
                    nc.tensor.matmul(
                        acc[:],
                        w_sb[:, k, j * P : (j + 1) * P],
                        hTsb[:, k, nsl],
                        start=(k == 0),
                        stop=(k == KT - 1),
                    )
                if half == 1:
                    nc.vector.tensor_scalar_add(
                        hi[:, j, nsl], acc[:], b_sb[:, j : j + 1]
                    )
                    nc.vector.scalar_tensor_tensor(
                        lo[:, j, nsl], acc[:], b_sb[:, j : j + 1],
                        hi[:, j, nsl], op0=ALU.add, op1=ALU.subtract,
                    )

            def v_group(s):
                # padded to [P, 512] so the tag size matches the other
                # "op" tiles; only [:, 0:R] is used
                acc = ps.tile([P, 512], F32, tag="op", bufs=2, name=f"vacc_{s}")
                for k in range(KT):
                    nc.tensor.matmul(
                        acc[:, 0:R],
                        hTsb[:, k, s * P : (s + 1) * P],
                        wvT[:, k, :],
                        start=(k == 0),
                        stop=False,
                    )
                nc.tensor.matmul(
                    acc[:, 0:R], ones_r[:], bv[:], start=False, stop=True
                )
                nc.scalar.copy(
                    vsb[:, s, :, 0:HD],
                    acc[:, 0:R].rearrange("p (h d) -> p h d", h=HG),
                )

            def kdup_pair(j, n):
                # k DoubleRow layout for head-pair j, s-chunk n: partitions
                # 0:64 = k_hi, 64:128 = k_lo, subtile dup via stride-0
                nsl = slice(n * 512, (n + 1) * 512)
                for h in (2 * j, 2 * j + 1):
                    hr = slice((h % 2) * HD, (h % 2) * HD + HD)
                    nc.sync.dma_start(km[0:HD, h, nsl], khi[hr, j, nsl])
                    nc.sync.dma_start(km[HD:P, h, nsl], klo[hr, j, nsl])

            def qdup_pair(j, hf):
                # q DoubleRow layout for head-pair j, t columns
                # [hf*1024, (hf+1)*1024): subtile 0 = [q_hi; q_hi], 1 = lo
                sl = slice(hf * 1024, (hf + 1) * 1024)
                for h in (2 * j, 2 * j + 1):
                    hr = slice((h % 2) * HD, (h % 2) * HD + HD)
                    for dst_half in range(2):
                        dsl = slice(dst_half * HD, dst_half * HD + HD)
                        nc.sync.dma_start(qm[dsl, h, 0, sl], qhi[hr, j, sl])
                        nc.sync.dma_start(qm[dsl, h, 1, sl], qlo[hr, j, sl])

            # ---------------- attention ----------------
            pending_pv = []
            deferred_pv = {}

            def attn_unit(h, tch, st, pv_pair, suppress_pv=False):
                """Scores (fp8 DR) -> exp -> f-mul for (head, t-chunk, s-tile);
                the PV matmuls are deferred PV_LAG units for PE pipelining.
                suppress_pv defers them entirely (early-start units whose pv
                slots are still held by an unfinished predecessor head)."""
                ssl = slice(st * P, (st + 1) * P)
                sc = ps.tile([P, TW], F32, tag="sc", bufs=2,
                             name=f"sc_{h}_{tch}_{st}")
                e = sb.tile([P, TW], BF16, tag="e", bufs=15,
                            name=f"e_{h}_{tch}_{st}")
                for a in range(TW // 512):
                    tsl = slice(tch * TW + a * 512, tch * TW + (a + 1) * 512)
                    nc.tensor.matmul(
                        sc[:, a * 512 : (a + 1) * 512],
                        km[:, h, ssl].unsqueeze(1).broadcast_to([P, 2, P]),
                        qm[:, h, :, tsl],
                        start=True,
                        stop=True,
                        perf_mode=DR,
                    )
                nc.scalar.activation(e[:], sc[:], AF.Exp, scale=1.0 / QSCALE)
                fmul_i[0] += 1
                # Pool takes a share of f-muls only inside the startup window
                # where DVE is the wall (units ~20-60); elsewhere Pool's slow
                # tiles would sit on the pv critical chain
                eng = (
                    nc.gpsimd
                    if (FMUL_GPS_MOD and 20 <= fmul_i[0] <= 60
                        and fmul_i[0] % FMUL_GPS_MOD == 0)
                    else nc.vector
                )
                eng.tensor_mul(
                    e[:], e[:], fT[:, st, tch * TW : (tch + 1) * TW]
                )
                if suppress_pv:
                    deferred_pv.setdefault((h, tch), []).append(
                        (h, st, e, pv_pair)
                    )
                else:
                    pending_pv.append((h, st, e, pv_pair))
                    while len(pending_pv) > PV_LAG:
                        flush_one_pv()

            def flush_one_pv():
                h, st, e, pv_pair = pending_pv.pop(0)
                for j4 in range(TW // P):
                    # start=True pending-zeroes the whole 2KB PSUM bank, so
                    # only the first slice of each pv bank may issue it; the
                    # other slices' first writes land on pending-zero bytes
                    # and are replaced (not accumulated) anyway.
                    nc.tensor.matmul(
                        pv_pair[j4 // 4][:, j4 % 4, :],
                        e[:, j4 * P : (j4 + 1) * P],
                        vsb[:, st, h, :],
                        start=(st == 0 and j4 % 4 == 0),
                        stop=(st == ST - 1),
                        skip_group_check=True,
                    )

            def flush_pv():
                while pending_pv:
                    flush_one_pv()

            def norm_head(h, tch, pvt, last=False):
                """recip of rowsum col; scalar-mul; PE transpose back to [d,t];
                copy into po.  For the final head (ACT idle) the muls split
                across ACT and DVE."""
                pv_pair, tp = pvt[:2], pvt[2]
                rc = sb.tile([P, 8], F32, tag="rc", bufs=2, name=f"rc_{h}_{tch}")
                for half in range(2):
                    nc.vector.reciprocal(
                        rc[:, half * 4 : half * 4 + 4],
                        pv_pair[half][:, :, HD : HD + 1].rearrange("p f one -> p (f one)"),
                    )
                # odd heads land on partitions 64:128 so the PSUM->SBUF copy
                # stays lane-aligned with po's row range
                pb = (h % 2) * HD
                psl = slice(pb, pb + HD)
                for half in range(2):
                    pn = sb.tile([P, 4, HD], BF16, tag="pn", bufs=3,
                                 name=f"pn_{h}_{tch}_{half}")
                    nc.vector.tensor_mul(
                        pn[:], pv_pair[half][:, :, 0:HD],
                        rc[:, half * 4 : half * 4 + 4]
                        .unsqueeze(-1).broadcast_to([P, 4, HD]),
                    )
                    for i in range(4):
                        nc.tensor.transpose(
                            tp[psl, half * 4 + i, :], pn[:, i, :], ident[:]
                        )
                if last:
                    for hh in range(2):
                        nc.vector.tensor_copy(
                            po[psl, h // 2,
                               tch * TW + hh * 512 : tch * TW + (hh + 1) * 512],
                            tp[psl, hh * 4 : hh * 4 + 4, :]
                            .rearrange("p a b -> p (a b)"),
                        )
                else:
                    nc.vector.tensor_copy(
                        po[psl, h // 2, tch * TW : (tch + 1) * TW],
                        tp[psl, :, :].rearrange("p a b -> p (a b)"),
                    )

            def new_pv_pair(h, tch):
                """PV accumulators + the transpose-staging tile, allocated
                together so the pv-tag slot rotation stays in lockstep."""
                pvt = [
                    ps.tile([P, 4, HD + 1], F32, tag="pv", bufs=2,
                            name=f"pv_{h}_{tch}_{i}")
                    for i in range(2)
                ]
                tp = ps.tile([P, 8, P], BF16, tag="pv", bufs=2,
                             name=f"tp_{h}_{tch}")
                pvt.append(tp)
                return pvt

            def outproj_unit(tt):
                osb = sb.tile([P, D], BF16, tag="osb", bufs=3, name=f"osb_{tt}")
                tail = tt >= 8
                for nh in range(2):
                    # tail fins alternate between the op tag and the freed
                    # pv banks so four accumulators are in flight
                    ftag = "pv" if (tail and tt % 2 == 1) else "op"
                    finn = ps.tile([P, 512], F32, tag=ftag, bufs=2,
                                   name=f"fin_{tt}_{nh}")
                    for j in range(MT):
                        nc.tensor.matmul(
                            finn[:],
                            po[:, j, tt * P : (tt + 1) * P],
                            woT[:, j, nh * 512 : (nh + 1) * 512],
                            start=(j == 0),
                            stop=(j == MT - 1),
                        )
                    # GPSIMD cannot read PSUM
                    if tail and nh == 0:
                        nc.scalar.copy(osb[:, 0:512], finn[:])
                    else:
                        nc.vector.tensor_copy(
                            osb[:, nh * 512 : (nh + 1) * 512], finn[:]
                        )
                nc.sync.dma_start(out_d[tt * P : (tt + 1) * P, :], osb[:])

            # ---------------- emission ----------------
            pv00 = new_pv_pair(0, 0)
            pv10 = new_pv_pair(1, 0)
            pv20 = new_pv_pair(2, 0)
            H1_EARLY, H2_EARLY = 6, 3

            # phase 0: everything h0's first attention units need (the pv
            # matmuls lag 2 units, so v streams just behind)
            hdma(0)
            hdma(1)
            for n in (0, 1):
                for qk in (0, 1):
                    for half in (0, 1):
                        qk_half(0, n, qk, half)
            kdup_pair(0, 0)
            kdup_pair(0, 1)
            qdup_pair(0, 0)
            # fT's 5.8us transfer would delay the dup DMAs (and so the first
            # scores) if queued earlier; the f-multiply it feeds is deferred
            # behind the PV lag anyway
            ft_piece(0)
            v_group(0)
            v_group(1)

            # stream: remaining QKV work in h0-unlock order, one attention
            # unit after each thunk when its gates have passed
            done = set()
            stream = []

            def ev(th, *events):
                stream.append((th, events))

            ev(lambda: v_group(2), "v2")
            ev(lambda: v_group(3), "v3")
            ev(lambda: v_group(4), "v4")
            ev(lambda: v_group(5), "v5")
            ev(lambda: ft_piece(1), "f1")
            ev(lambda: v_group(6), "v6")
            ev(lambda: v_group(7), "v7")
            ev(lambda: hdma(2))
            ev(lambda: qk_half(0, 2, 1, 0))
            ev(lambda: qk_half(0, 2, 1, 1))
            ev(lambda: kdup_pair(0, 2), "kd2")
            ev(lambda: ft_piece(2), "f2")
            ev(lambda: v_group(8), "v8")
            ev(lambda: v_group(9), "v9")
            ev(lambda: v_group(10), "v10")
            ev(lambda: v_group(11), "v11")
            ev(lambda: hdma(3))
            ev(lambda: qk_half(0, 3, 1, 0))
            ev(lambda: qk_half(0, 3, 1, 1))
            ev(lambda: kdup_pair(0, 3), "kd3")
            ev(lambda: ft_piece(3), "f3")
            ev(lambda: v_group(12), "v12")
            ev(lambda: v_group(13), "v13")
            ev(lambda: v_group(14), "v14")
            ev(lambda: v_group(15), "v15")
            for n in (2, 3):
                for hf in (0, 1):
                    ev(lambda n=n, hf=hf: qk_half(0, n, 0, hf))
            ev(lambda: qdup_pair(0, 1))
            for n in range(NCH):
                for qk in (0, 1):
                    for hf in (0, 1):
                        ev(lambda n=n, qk=qk, hf=hf: qk_half(1, n, qk, hf))
                ev(lambda n=n: kdup_pair(1, n), f"kd1_{n}")
                if n == 1:
                    ev(lambda: qdup_pair(1, 0), "qd1")
            ev(lambda: qdup_pair(1, 1))
            ev(lambda: nc.sync.dma_start(
                woT[:], woT_d.rearrange("(m p) d -> p m d", p=P)))

            def h0_ready(st):
                # the pv matmuls for unit st are emitted PV_LAG units later,
                # so v only needs to be a few steps ahead of the flush
                need = []
                vst = st - (PV_LAG - 1)
                if vst >= 2:
                    need.append(f"v{vst}")
                if st >= 8:
                    need.append("kd2" if st < 12 else "kd3")
                if st >= 4:
                    need.append(f"f{st // 4}")
                return all(x in done for x in need)

            def h2_ready(st):
                return all(x in done for x in ("qd1", f"kd1_{st // 4}"))

            ui0, ui1, ui2 = 0, 0, 0
            for th, events in stream:
                # emit the eligible unit BEFORE the thunk so its scores sit
                # ahead of the thunk's matmuls in the PE queue
                if ui0 < ST and h0_ready(ui0):
                    attn_unit(0, 0, ui0, pv00)
                    ui0 += 1
                elif ui0 >= ST and ui1 < H1_EARLY:
                    attn_unit(1, 0, ui1, pv10, suppress_pv=True)
                    ui1 += 1
                elif (ui0 >= ST and ui1 >= H1_EARLY and ui2 < H2_EARLY
                        and h2_ready(ui2)):
                    attn_unit(2, 0, ui2, pv20, suppress_pv=True)
                    ui2 += 1
            while ui0 < ST:
                attn_unit(0, 0, ui0, pv00)
                ui0 += 1

            op_next = [0]

            def emit_outproj(kmax):
                while op_next[0] < kmax:
                    outproj_unit(op_next[0])
                    op_next[0] += 1

            def start_head(h, tch, pvp, first_st, prev3):
                """Emit this head's first unit (pv deferred), drain and norm
                the previous head, then adopt the deferred backlog so it
                flushes behind the norm."""
                attn_unit(h, tch, first_st, pvp, suppress_pv=True)
                while pending_pv:
                    flush_one_pv()
                norm_head(*prev3)
                pending_pv.extend(deferred_pv.pop((h, tch), []))

            while ui1 < ST:
                attn_unit(1, 0, ui1, pv10)
                ui1 += 1
            prev3 = (1, 0, pv10)
            starts = {(0, 2): ui2}
            pairs = {(0, 2): pv20}
            for tch in range(TCH):
                for h in range(HG):
                    if tch == 0 and h <= 1:
                        continue
                    pvp = pairs.get((tch, h)) or new_pv_pair(h, tch)
                    first = starts.get((tch, h), 0)
                    start_head(h, tch, pvp, first, prev3)
                    for st in range(first + 1, ST):
                        attn_unit(h, tch, st, pvp)
                        if tch == 1 and h == 3:
                            # drain eagerly so the post-exp tail is short
                            while len(pending_pv) > 2:
                                flush_one_pv()
                        if tch == 1 and h < 3 and (h * ST + st) % 5 == 4:
                            # tt 0..7 (tch0) spread over h0..h2 of tch1 --
                            # h3's window must stay clean so the last exps
                            # aren't interleaved with out-proj traffic
                            emit_outproj(min(8, (h * ST + st) // 5 + 1))
                    prev3 = (h, tch, pvp)
            flush_pv()
            norm_head(3, 1, prev3[2], last=True)
            emit_outproj(16)

    return nc


_NC = None
_LAST_RESULT = None


def _get_nc():
    global _NC
    if _NC is None:
        _NC = build_bass()
        if not _NC.is_finalized():
            _NC.finalize()
    return _NC


def kernel(hidden_states, focused_attention, Wq, bq, Wk, bk, Wv, bv, Wo, bo):
    bf = ml_dtypes.bfloat16
    hT = [np.ascontiguousarray(hidden_states[b].T).astype(bf) for b in range(B)]
    fT = [np.ascontiguousarray(focused_attention[b].T).astype(bf) for b in range(B)]

    in_maps = []
    for c in range(N_CORES):
        b, g = divmod(c, 4)
        rows = slice(g * R, (g + 1) * R)
        in_maps.append({
            "hT": hT[b],
            "fT": fT[b],
            "wqT": np.ascontiguousarray((Wq[rows] * (SCALING * QSCALE)).T).astype(bf),
            "wkT": np.ascontiguousarray(Wk[rows].T).astype(bf),
            "wvT": np.ascontiguousarray(Wv[rows].T).astype(bf),
            "woT": np.ascontiguousarray(Wo[:, rows].T).astype(bf),
            "bq": np.ascontiguousarray(
                (bq[rows] * (SCALING * QSCALE))[:, None]
            ).astype(np.float32),
            "bk": np.ascontiguousarray(bk[rows][:, None]).astype(np.float32),
            "bv": np.ascontiguousarray(bv[rows][None, :]).astype(bf),
        })

    res = run_bass_kernel_spmd(_get_nc(), in_maps, list(range(N_CORES)))
    global _LAST_RESULT
    _LAST_RESULT = res
    out = np.zeros((B, T, D), dtype=np.float32)
    for c in range(N_CORES):
        out[c // 4] += np.asarray(res.results[c]["out_partial"], dtype=np.float32)
    out += np.asarray(bo, dtype=np.float32)[None, None, :]
    return out
